# revision 20
# baseline (speedup 1.0000x reference)
"""Trainium2 Bass kernel: fused multi-head attention (QKV proj + RoPE +
softmax attention + output projection).

Problem dims: x[B=2, S=2048, H=1024], 16 heads, head_dim 64, fp32.

Sharding (8 NeuronCores): core = (batch b, head-group g); each core owns
batch b and 4 heads [4g..4g+4). It computes, fully on-device:
  - qkv projection for its heads (q/k produced feature-major, v seq-major)
  - RoPE on q/k
  - scoresT = k_rope^T-layout scores, exp (with mask bias + 1/sqrt(d) scale)
  - context via exp-scores @ v with an appended ones-column that yields the
    softmax denominators for free; per-query normalization
  - output projection against its 256 rows of w_out -> partial [S, 1024]
Host: shards/casts inputs per core, then sums the 4 per-batch partials.

The kernel is self-contained: call kernel(**inputs) with the full unsharded
inputs from setup_inputs(); returns the full [2, 2048, 1024] fp32 output.
"""

import math
import os
import sys
from dataclasses import dataclass

import numpy as np

for _p in ("/root/.axon_site/_ro/trn_rl_repo", "/opt/trn_rl_repo"):
    if _p not in sys.path and os.path.isdir(_p):
        sys.path.append(_p)

# Problem constants (hardcoded per spec; do not read spec.json at runtime).
B = 2
S_FULL = 2048
H_FULL = 1024
NUM_HEADS = 16
D = 64  # head dim
N_CORES = 8
GROUPS = N_CORES // B  # head groups per batch = 4
HPC = NUM_HEADS // GROUPS  # heads per core = 4 (2 pairs)

# Matmul operand dtype: "bf16" (fast), "fp32" (exact), "fp32r" (middle).
MM_DTYPE = os.environ.get("KERNEL_MM_DTYPE", "bf16")


@dataclass(frozen=True)
class Cfg:
    S: int = S_FULL
    H: int = H_FULL
    mm_dtype: str = MM_DTYPE

    @property
    def NT(self):  # 128-wide seq tiles (key tiles / s tiles)
        return self.S // 128

    @property
    def QC(self):  # query-chunk width (matmul N)
        return min(512, self.S)

    @property
    def NQC(self):
        return self.S // self.QC

    @property
    def HT(self):  # hidden contraction tiles
        return self.H // 128

    @property
    def OC(self):  # out-proj N chunk
        return min(512, self.H)

    @property
    def NOC(self):
        return self.H // self.OC


def _emit(tc, t, cfg, stop_after=None):
    """Emit the per-core program. `t` maps dram tensor name -> AP."""
    import concourse.bass as bass
    from concourse import mybir

    nc = tc.nc
    f32 = mybir.dt.float32
    dmm = {"bf16": mybir.dt.bfloat16, "fp32": f32, "fp32r": f32}[cfg.mm_dtype]

    if cfg.mm_dtype == "fp32r":
        mm = lambda ap: ap.bitcast(mybir.dt.float32r)
    else:
        mm = lambda ap: ap

    S, NT, QC, NQC, HT, OC, NOC = (
        cfg.S, cfg.NT, cfg.QC, cfg.NQC, cfg.HT, cfg.OC, cfg.NOC)
    Exp = mybir.ActivationFunctionType.Exp
    mult = mybir.AluOpType.mult
    add = mybir.AluOpType.add

    import contextlib
    es = contextlib.ExitStack()
    with es:
        consts = es.enter_context(tc.tile_pool(name="consts", bufs=1))
        xpool = es.enter_context(tc.tile_pool(name="xpool", bufs=2))
        store = es.enter_context(tc.tile_pool(name="store", bufs=1))
        rot_pool = es.enter_context(tc.tile_pool(name="rot", bufs=2))
        exp_pool = es.enter_context(tc.tile_pool(name="expp", bufs=4))
        rc_pool = es.enter_context(tc.tile_pool(name="rcp", bufs=3))
        ctxu_pool = es.enter_context(tc.tile_pool(name="ctxu", bufs=3))
        out_pool = es.enter_context(tc.tile_pool(name="outp", bufs=2))
        ps_big = es.enter_context(tc.tile_pool(name="ps_big", bufs=2, space="PSUM"))
        ps_ctx = es.enter_context(tc.tile_pool(name="ps_ctx", bufs=3, space="PSUM"))
        ps_rep = es.enter_context(tc.tile_pool(name="ps_rep", bufs=1, space="PSUM"))

        # ---- constants / weights to SBUF ----
        # Order matters: the first x-chunk + wqk unblock the first matmuls.
        wqk_sb = consts.tile([128, HT, 4 * 128], dmm)
        for ht in range(HT):
            nc.sync.dma_start(out=wqk_sb[:, ht, :], in_=t["wqk"][ht * 128:(ht + 1) * 128, :])
        x0 = xpool.tile([128, HT, QC], dmm, tag="xt", name="xt_sc0")
        for ht in range(HT):
            nc.sync.dma_start(out=x0[:, ht, :], in_=t["xT"][ht * 128:(ht + 1) * 128, 0:QC])
        cos_sb = consts.tile([128, S], f32)
        nc.sync.dma_start(out=cos_sb, in_=t["cosT"])
        sin_sb = consts.tile([128, S], f32)
        nc.sync.dma_start(out=sin_sb, in_=t["sinS"])
        mb_sb = consts.tile([128, NT], f32)
        nc.sync.dma_start(out=mb_sb, in_=t["mbias"])
        wv_sb = consts.tile([128, HT, HPC * D], dmm)
        for ht in range(HT):
            nc.sync.dma_start(out=wv_sb[:, ht, :], in_=t["wv"][ht * 128:(ht + 1) * 128, :])
        wo_sb = consts.tile([128, 2, cfg.H], dmm)
        for ft in range(2):
            nc.sync.dma_start(out=wo_sb[:, ft, :], in_=t["wo"][ft * 128:(ft + 1) * 128, :])
        ones_sb = consts.tile([1, D], f32)
        nc.vector.memset(ones_sb, 1.0)

        # ---- persistent activations ----
        # qk_sb f-tiles: 0 = q pair0 (heads 0,1), 1 = q pair1 (heads 2,3),
        #                2 = k pair0,              3 = k pair1
        qk_sb = store.tile([128, 4, S], dmm)
        v_sb = store.tile([128, NT, HPC, D + 1], dmm)
        cT_sb = store.tile([128, 2, S], dmm)

        # ---- phase 1: qkv projection (+ rope, + v staging) ----
        for sc in range(NQC):
            qs = slice(sc * QC, (sc + 1) * QC)
            if sc == 0:
                xt = x0
            else:
                xt = xpool.tile([128, HT, QC], dmm, tag="xt", name=f"xt_sc{sc}")
                for ht in range(HT):
                    nc.sync.dma_start(out=xt[:, ht, :],
                                      in_=t["xT"][ht * 128:(ht + 1) * 128, qs])
            for f in range(4):
                ps = ps_ctx.tile([128, QC], f32, tag="ctx", name=f"qkps{sc}_{f}")
                for ht in range(HT):
                    nc.tensor.matmul(
                        ps, lhsT=mm(wqk_sb[:, ht, f * 128:(f + 1) * 128]),
                        rhs=mm(xt[:, ht, :]),
                        start=(ht == 0), stop=(ht == HT - 1))
                # Free the PSUM bank fast via an ACT copy (ACT is idle in this
                # phase), then run RoPE on SBUF where DVE gets 2x mode:
                #   qk = raw * cos + shift32(raw) * sin_signed
                # DVE requires equal input base partitions, so sin_sb holds a
                # 32-block-swapped table (sinSh[sq] == sin_signed[dq]) and the
                # quadrant crossing happens only on the output write.
                raw = rot_pool.tile([128, QC], f32, tag="raw")
                nc.scalar.copy(raw, ps)
                tr = rot_pool.tile([128, QC], f32, tag="tr")
                for dq, sq in ((0, 32), (32, 0), (64, 96), (96, 64)):
                    nc.vector.tensor_tensor(
                        out=tr[dq:dq + 32, :], in0=raw[sq:sq + 32, :],
                        in1=sin_sb[sq:sq + 32, qs], op=mult)
                qk_slice = qk_sb[:, f, qs]
                nc.vector.tensor_tensor(out=qk_slice, in0=raw, in1=cos_sb[:, qs], op=mult)
                nc.vector.tensor_tensor(out=qk_slice, in0=qk_slice, in1=tr, op=add)
            # v for the s-tiles inside this chunk (seq-major, + ones col)
            for stl in range(QC // 128):
                st = sc * (QC // 128) + stl
                psv = ps_ctx.tile([128, HPC * D], f32, tag="ctx")
                for ht in range(HT):
                    nc.tensor.matmul(
                        psv, lhsT=mm(xt[:, ht, stl * 128:(stl + 1) * 128]),
                        rhs=mm(wv_sb[:, ht, :]),
                        start=(ht == 0), stop=(ht == HT - 1))
                nc.vector.tensor_copy(
                    v_sb[:, st, :, 0:D],
                    psv.rearrange("p (h d) -> p h d", h=HPC))
                nc.vector.memset(v_sb[:, st, :, D:D + 1], 1.0)

        if "dbg_qk" in t:
            nc.sync.dma_start(out=t["dbg_qk"], in_=qk_sb)
            nc.sync.dma_start(out=t["dbg_v"], in_=v_sb)
        if stop_after == "qkv":
            return

        # ---- phase 2+3: attention with fused out-projection per q-chunk ----
        inv_sqrt_d = 1.0 / math.sqrt(D)
        for qc in range(NQC):
            qs = slice(qc * QC, (qc + 1) * QC)
            for p in range(2):
                hA, hB = 2 * p, 2 * p + 1
                ctx_ps = [ps_ctx.tile([D + 1, QC], f32, tag="ctx", name=f"ctxps{p}_{qc}_{i}")
                          for i in range(2)]
                for kt in range(NT):
                    ks = slice(kt * 128, (kt + 1) * 128)
                    sc_ps = ps_big.tile([128, 2 * QC], f32, tag="big")
                    for i, pb in enumerate((0, 64)):
                        nc.tensor.matmul(
                            sc_ps[:, i * QC:(i + 1) * QC],
                            lhsT=mm(qk_sb[pb:pb + 64, 2 + p, ks]),
                            rhs=mm(qk_sb[pb:pb + 64, p, qs]),
                            start=True, stop=True)
                    ex = exp_pool.tile([128, 2 * QC], dmm, tag="expT")
                    nc.scalar.activation(ex, sc_ps, Exp,
                                         bias=mb_sb[:, kt:kt + 1], scale=inv_sqrt_d)
                    for i, h in enumerate((hA, hB)):
                        nc.tensor.matmul(
                            ctx_ps[i], lhsT=mm(v_sb[:, kt, h, :]),
                            rhs=mm(ex[:, i * QC:(i + 1) * QC]),
                            start=(kt == 0), stop=(kt == NT - 1))
                # normalize: cT = ctx[0:D] * (1/denom) broadcast over partitions.
                # Stage ctx psum to SBUF right away so the PSUM bank frees for
                # the next q-chunk; recip/replicate/mult run off-critical-path.
                for i, h in enumerate((hA, hB)):
                    ctxu = ctxu_pool.tile([D, QC], f32, tag="ctxu",
                                          name=f"ctxu{p}_{qc}_{i}")
                    nc.vector.tensor_copy(ctxu, ctx_ps[i][0:D, :])
                    # custom-DVE recip requires partition-0-based input: stage
                    # the denominator row to a base-0 tile first.
                    den = rc_pool.tile([1, QC], f32, tag="den")
                    nc.vector.tensor_copy(den, ctx_ps[i][D:D + 1, :])
                    rc = rc_pool.tile([1, QC], f32, tag="rc")
                    if cfg.mm_dtype == "bf16":
                        nc.vector.reciprocal_approx_fast(rc, den)
                    else:
                        rcs = rc_pool.tile([1, QC], f32, tag="rcs")
                        nc.vector.reciprocal_approx_accurate(rc, den, scratch=rcs)
                    rep = ps_rep.tile([D, QC], f32, tag="rep")
                    nc.tensor.matmul(rep, lhsT=ones_sb, rhs=rc, start=True, stop=True)
                    # in0 SBUF + in1 PSUM: single PSUM read operand.
                    nc.vector.tensor_tensor(
                        out=cT_sb[(h % 2) * D:(h % 2) * D + D, p, qs],
                        in0=ctxu[0:D, :], in1=rep, op=mult)
            if stop_after == "attn":
                continue
            # out-projection for this q-chunk's s-tiles (overlaps the next
            # q-chunk's exp work on ACT)
            for stl in range(QC // 128):
                st = qc * (QC // 128) + stl
                ss = slice(st * 128, (st + 1) * 128)
                ot = out_pool.tile([128, cfg.H], f32, tag="ot")
                for oc in range(NOC):
                    po = ps_big.tile([128, OC], f32, tag="big")
                    for ft in range(2):
                        nc.tensor.matmul(
                            po, lhsT=mm(cT_sb[:, ft, ss]),
                            rhs=mm(wo_sb[:, ft, oc * OC:(oc + 1) * OC]),
                            start=(ft == 0), stop=(ft == 1))
                    nc.vector.tensor_copy(ot[:, oc * OC:(oc + 1) * OC], po)
                nc.sync.dma_start(out=t["out"][ss, :], in_=ot)

        if "dbg_cT" in t:
            nc.sync.dma_start(out=t["dbg_cT"], in_=cT_sb)


def build(cfg: Cfg, dbg=False, stop_after=None):
    """Build + compile the per-core program. Returns (nc, input names)."""
    import concourse.tile as tile
    from concourse import bacc, mybir

    f32 = mybir.dt.float32
    dmm = {"bf16": mybir.dt.bfloat16, "fp32": f32, "fp32r": f32}[cfg.mm_dtype]

    nc = bacc.Bacc("TRN2", debug=False, enable_asserts=False,
                   target_bir_lowering=False)
    t = {}
    t["xT"] = nc.dram_tensor("xT", [cfg.H, cfg.S], dmm, kind="ExternalInput").ap()
    t["wqk"] = nc.dram_tensor("wqk", [cfg.H, 4 * 128], dmm, kind="ExternalInput").ap()
    t["wv"] = nc.dram_tensor("wv", [cfg.H, HPC * D], dmm, kind="ExternalInput").ap()
    t["wo"] = nc.dram_tensor("wo", [HPC * D, cfg.H], dmm, kind="ExternalInput").ap()
    t["cosT"] = nc.dram_tensor("cosT", [128, cfg.S], f32, kind="ExternalInput").ap()
    t["sinS"] = nc.dram_tensor("sinS", [128, cfg.S], f32, kind="ExternalInput").ap()
    t["mbias"] = nc.dram_tensor("mbias", [128, cfg.NT], f32, kind="ExternalInput").ap()
    t["out"] = nc.dram_tensor("out", [cfg.S, cfg.H], f32, kind="ExternalOutput").ap()
    if dbg:
        t["dbg_qk"] = nc.dram_tensor(
            "dbg_qk", [128, 4, cfg.S], dmm, kind="ExternalOutput").ap()
        t["dbg_v"] = nc.dram_tensor(
            "dbg_v", [128, cfg.NT, HPC, D + 1], dmm, kind="ExternalOutput").ap()
        t["dbg_cT"] = nc.dram_tensor(
            "dbg_cT", [128, 2, cfg.S], dmm, kind="ExternalOutput").ap()

    with tile.TileContext(nc) as tc:
        _emit(tc, t, cfg, stop_after=stop_after)
    nc.compile()
    return nc


# ----------------------------------------------------------------------------
# Host side: shard, run, gather
# ----------------------------------------------------------------------------

def rope_tables(S, dtype=np.float32):
    """cos/sin tables exactly as the reference builds them."""
    inv_freq = 1.0 / (10000.0 ** (np.arange(0, D, 2, dtype=np.float32) / D))
    tt = np.arange(S, dtype=np.float32)
    freqs = np.einsum("i,j->ij", tt, inv_freq)  # [S, D/2]
    emb = np.concatenate([freqs, freqs], axis=-1)  # [S, D]
    return np.cos(emb).astype(dtype), np.sin(emb).astype(dtype)


def device_rope_tables(S):
    """(cosT, sinSh) in the [128, S] partition layout the kernel expects.

    cosT: cos table transposed, stacked twice (two heads per 128 partitions).
    sinSh: sign-folded sin table (rotate_half sign), transposed, stacked, and
    32-block swapped so the rope multiply reads it at the SOURCE partition
    base (walrus requires equal input base partitions on DVE tensor ops).
    """
    cos, sin = rope_tables(S)
    cosT = np.ascontiguousarray(np.tile(cos.T, (2, 1)))  # [128, S]
    sinT = sin.T.copy()  # [D, S]
    sinT[:D // 2, :] *= -1.0
    sinS = np.tile(sinT, (2, 1))  # [128, S], signed
    perm = np.r_[32:64, 0:32, 96:128, 64:96]
    sinSh = np.ascontiguousarray(sinS[perm])
    return cosT.astype(np.float32), sinSh.astype(np.float32)


def make_in_maps(x, attention_mask, w_qkv, w_out, cfg: Cfg):
    """Build the 8 per-core input maps (numpy prep only)."""
    import ml_dtypes
    np_mm = {"bf16": np.dtype(ml_dtypes.bfloat16),
             "fp32": np.float32, "fp32r": np.float32}[cfg.mm_dtype]

    S, H = cfg.S, cfg.H
    cosT, sinS = device_rope_tables(S)

    in_maps = []
    for core in range(N_CORES):
        b, g = core // GROUPS, core % GROUPS
        heads = [g * HPC + j for j in range(HPC)]  # global head ids
        # Reference reshapes qkv to [B,S,16,192]: head h owns w_qkv columns
        # [h*3D, (h+1)*3D) as [q (D) | k (D) | v (D)].
        qcols = np.concatenate([np.arange(h * 3 * D, h * 3 * D + D) for h in heads])
        kcols = np.concatenate([np.arange(h * 3 * D + D, h * 3 * D + 2 * D) for h in heads])
        vcols = np.concatenate([np.arange(h * 3 * D + 2 * D, h * 3 * D + 3 * D) for h in heads])
        wqk = np.ascontiguousarray(
            np.concatenate([w_qkv[:, qcols], w_qkv[:, kcols]], axis=1)).astype(np_mm)
        wv = np.ascontiguousarray(w_qkv[:, vcols]).astype(np_mm)
        wo = np.ascontiguousarray(
            w_out[heads[0] * D:(heads[-1] + 1) * D, :]).astype(np_mm)
        xT = np.ascontiguousarray(np.asarray(x)[b].T).astype(np_mm)  # [H, S]
        mask = np.asarray(attention_mask)[b].astype(np.float32)  # [S]
        mbias = ((mask - 1.0) * 3e38).reshape(cfg.NT, 128).T.copy()  # [128, NT]
        in_maps.append({
            "xT": xT, "wqk": wqk, "wv": wv, "wo": wo,
            "cosT": cosT, "sinS": sinS, "mbias": mbias.astype(np.float32),
        })
    return in_maps


_PROG_CACHE = {}


def get_program(cfg: Cfg):
    if cfg not in _PROG_CACHE:
        _PROG_CACHE[cfg] = build(cfg)
    return _PROG_CACHE[cfg]


def run_sharded(x, attention_mask, w_qkv, w_out, trace=False, cfg: Cfg = None):
    """Run on the 8 NeuronCores; returns (full_output, BassKernelResults)."""
    from concourse import bass_utils

    cfg = cfg or Cfg()
    nc = get_program(cfg)
    in_maps = make_in_maps(x, attention_mask, w_qkv, w_out, cfg)
    res = bass_utils.run_bass_kernel_spmd(
        nc, in_maps, core_ids=list(range(N_CORES)), trace=trace)
    parts = [r["out"].astype(np.float64) for r in res.results]
    out = np.stack([
        sum(parts[b * GROUPS + g] for g in range(GROUPS))
        for b in range(B)
    ]).astype(np.float32)  # [B, S, H]
    return out, res


def kernel(x=None, attention_mask=None, w_qkv=None, w_out=None, **_ignored):
    out, _ = run_sharded(x, attention_mask, w_qkv, w_out, trace=False)
    return out


# revision 25
# speedup vs baseline: 1.1029x; 1.1029x over previous
"""Trainium2 Bass kernel: fused multi-head attention (QKV proj + RoPE +
softmax attention + output projection).

Problem dims: x[B=2, S=2048, H=1024], 16 heads, head_dim 64, fp32.

Sharding (8 NeuronCores): core = (batch b, head-group g); each core owns
batch b and 4 heads [4g..4g+4). It computes, fully on-device:
  - qkv projection for its heads (q/k produced feature-major, v seq-major)
  - RoPE on q/k
  - scoresT = k_rope^T-layout scores, exp (with mask bias + 1/sqrt(d) scale)
  - context via exp-scores @ v with an appended ones-column that yields the
    softmax denominators for free; per-query normalization
  - output projection against its 256 rows of w_out -> partial [S, 1024]
Host: shards/casts inputs per core, then sums the 4 per-batch partials.

The kernel is self-contained: call kernel(**inputs) with the full unsharded
inputs from setup_inputs(); returns the full [2, 2048, 1024] fp32 output.
"""

import math
import os
import sys
from dataclasses import dataclass

import numpy as np

for _p in ("/root/.axon_site/_ro/trn_rl_repo", "/opt/trn_rl_repo"):
    if _p not in sys.path and os.path.isdir(_p):
        sys.path.append(_p)

# Problem constants (hardcoded per spec; do not read spec.json at runtime).
B = 2
S_FULL = 2048
H_FULL = 1024
NUM_HEADS = 16
D = 64  # head dim
N_CORES = 8
GROUPS = N_CORES // B  # head groups per batch = 4
HPC = NUM_HEADS // GROUPS  # heads per core = 4 (2 pairs)

# Matmul operand dtype: "bf16" (fast), "fp32" (exact), "fp32r" (middle).
MM_DTYPE = os.environ.get("KERNEL_MM_DTYPE", "bf16")


@dataclass(frozen=True)
class Cfg:
    S: int = S_FULL
    H: int = H_FULL
    mm_dtype: str = MM_DTYPE

    @property
    def NT(self):  # 128-wide seq tiles (key tiles / s tiles)
        return self.S // 128

    @property
    def QC(self):  # query-chunk width (matmul N)
        return min(512, self.S)

    @property
    def NQC(self):
        return self.S // self.QC

    @property
    def HT(self):  # hidden contraction tiles
        return self.H // 128

    @property
    def OC(self):  # out-proj N chunk
        return min(512, self.H)

    @property
    def NOC(self):
        return self.H // self.OC


def _emit(tc, t, cfg, stop_after=None):
    """Emit the per-core program. `t` maps dram tensor name -> AP."""
    import concourse.bass as bass
    from concourse import mybir

    nc = tc.nc
    f32 = mybir.dt.float32
    dmm = {"bf16": mybir.dt.bfloat16, "fp32": f32, "fp32r": f32}[cfg.mm_dtype]

    if cfg.mm_dtype == "fp32r":
        mm = lambda ap: ap.bitcast(mybir.dt.float32r)
    else:
        mm = lambda ap: ap

    S, NT, QC, NQC, HT, OC, NOC = (
        cfg.S, cfg.NT, cfg.QC, cfg.NQC, cfg.HT, cfg.OC, cfg.NOC)
    Exp = mybir.ActivationFunctionType.Exp
    mult = mybir.AluOpType.mult
    add = mybir.AluOpType.add

    import contextlib
    es = contextlib.ExitStack()
    with es:
        consts = es.enter_context(tc.tile_pool(name="consts", bufs=1))
        xpool = es.enter_context(tc.tile_pool(name="xpool", bufs=2))
        store = es.enter_context(tc.tile_pool(name="store", bufs=1))
        rot_pool = es.enter_context(tc.tile_pool(name="rot", bufs=2))
        exp_pool = es.enter_context(tc.tile_pool(name="expp", bufs=4))
        rc_pool = es.enter_context(tc.tile_pool(name="rcp", bufs=3))
        ctxu_pool = es.enter_context(tc.tile_pool(name="ctxu", bufs=3))
        out_pool = es.enter_context(tc.tile_pool(name="outp", bufs=2))
        ps_big = es.enter_context(tc.tile_pool(name="ps_big", bufs=2, space="PSUM"))
        ps_ctx = es.enter_context(tc.tile_pool(name="ps_ctx", bufs=3, space="PSUM"))

        # ---- constants / weights to SBUF ----
        # Order matters: the first x-chunk + wqk unblock the first matmuls.
        wqk_sb = consts.tile([128, HT, 4 * 128], dmm)
        for ht in range(HT):
            nc.sync.dma_start(out=wqk_sb[:, ht, :], in_=t["wqk"][ht * 128:(ht + 1) * 128, :])
        x0 = xpool.tile([128, HT, QC], dmm, tag="xt", name="xt_sc0")
        for ht in range(HT):
            nc.sync.dma_start(out=x0[:, ht, :], in_=t["xT"][ht * 128:(ht + 1) * 128, 0:QC])
        cos_sb = consts.tile([128, S], f32)
        nc.sync.dma_start(out=cos_sb, in_=t["cosT"])
        sin_sb = consts.tile([128, S], f32)
        nc.sync.dma_start(out=sin_sb, in_=t["sinS"])
        mb_sb = consts.tile([128, NT], f32)
        nc.sync.dma_start(out=mb_sb, in_=t["mbias"])
        wv_sb = consts.tile([128, HT, HPC * D], dmm)
        for ht in range(HT):
            nc.sync.dma_start(out=wv_sb[:, ht, :], in_=t["wv"][ht * 128:(ht + 1) * 128, :])
        wo_sb = consts.tile([128, 2, cfg.H], dmm)
        for ft in range(2):
            nc.sync.dma_start(out=wo_sb[:, ft, :], in_=t["wo"][ft * 128:(ft + 1) * 128, :])

        # ---- persistent activations ----
        # qk_sb f-tiles: 0 = q pair0 (heads 0,1), 1 = q pair1 (heads 2,3),
        #                2 = k pair0,              3 = k pair1
        qk_sb = store.tile([128, 4, S], dmm)
        v_sb = store.tile([128, NT, HPC, D + 1], dmm)
        cT_sb = store.tile([128, 2, S], dmm)

        # ---- phase 1: qkv projection (+ rope, + v staging) ----
        for sc in range(NQC):
            qs = slice(sc * QC, (sc + 1) * QC)
            if sc == 0:
                xt = x0
            else:
                xt = xpool.tile([128, HT, QC], dmm, tag="xt", name=f"xt_sc{sc}")
                for ht in range(HT):
                    nc.sync.dma_start(out=xt[:, ht, :],
                                      in_=t["xT"][ht * 128:(ht + 1) * 128, qs])
            for f in range(4):
                ps = ps_ctx.tile([128, QC], f32, tag="ctx", name=f"qkps{sc}_{f}")
                for ht in range(HT):
                    nc.tensor.matmul(
                        ps, lhsT=mm(wqk_sb[:, ht, f * 128:(f + 1) * 128]),
                        rhs=mm(xt[:, ht, :]),
                        start=(ht == 0), stop=(ht == HT - 1))
                # Free the PSUM bank fast via an ACT copy (ACT is idle in this
                # phase), then run RoPE on SBUF where DVE gets 2x mode:
                #   qk = raw * cos + shift32(raw) * sin_signed
                # DVE requires equal input base partitions, so sin_sb holds a
                # 32-block-swapped table (sinSh[sq] == sin_signed[dq]) and the
                # quadrant crossing happens only on the output write.
                raw = rot_pool.tile([128, QC], f32, tag="raw")
                nc.scalar.copy(raw, ps)
                tr = rot_pool.tile([128, QC], f32, tag="tr")
                for dq, sq in ((0, 32), (32, 0), (64, 96), (96, 64)):
                    nc.vector.tensor_tensor(
                        out=tr[dq:dq + 32, :], in0=raw[sq:sq + 32, :],
                        in1=sin_sb[sq:sq + 32, qs], op=mult)
                qk_slice = qk_sb[:, f, qs]
                nc.vector.tensor_tensor(out=qk_slice, in0=raw, in1=cos_sb[:, qs], op=mult)
                nc.vector.tensor_tensor(out=qk_slice, in0=qk_slice, in1=tr, op=add)
            # v for the s-tiles inside this chunk (seq-major, + ones col)
            for stl in range(QC // 128):
                st = sc * (QC // 128) + stl
                psv = ps_ctx.tile([128, HPC * D], f32, tag="ctx")
                for ht in range(HT):
                    nc.tensor.matmul(
                        psv, lhsT=mm(xt[:, ht, stl * 128:(stl + 1) * 128]),
                        rhs=mm(wv_sb[:, ht, :]),
                        start=(ht == 0), stop=(ht == HT - 1))
                nc.vector.tensor_copy(
                    v_sb[:, st, :, 0:D],
                    psv.rearrange("p (h d) -> p h d", h=HPC))
                nc.vector.memset(v_sb[:, st, :, D:D + 1], 1.0)

        if "dbg_qk" in t:
            nc.sync.dma_start(out=t["dbg_qk"], in_=qk_sb)
            nc.sync.dma_start(out=t["dbg_v"], in_=v_sb)
        if stop_after == "qkv":
            return

        # ---- phase 2+3: attention with fused out-projection per q-chunk ----
        inv_sqrt_d = 1.0 / math.sqrt(D)
        for qc in range(NQC):
            qs = slice(qc * QC, (qc + 1) * QC)
            for p in range(2):
                hA, hB = 2 * p, 2 * p + 1
                ctx_ps = [ps_ctx.tile([D + 1, QC], f32, tag="ctx", name=f"ctxps{p}_{qc}_{i}")
                          for i in range(2)]
                for kt in range(NT):
                    ks = slice(kt * 128, (kt + 1) * 128)
                    sc_ps = ps_big.tile([128, 2 * QC], f32, tag="big")
                    for i, pb in enumerate((0, 64)):
                        nc.tensor.matmul(
                            sc_ps[:, i * QC:(i + 1) * QC],
                            lhsT=mm(qk_sb[pb:pb + 64, 2 + p, ks]),
                            rhs=mm(qk_sb[pb:pb + 64, p, qs]),
                            start=True, stop=True)
                    ex = exp_pool.tile([128, 2 * QC], dmm, tag="expT")
                    nc.scalar.activation(ex, sc_ps, Exp,
                                         bias=mb_sb[:, kt:kt + 1], scale=inv_sqrt_d)
                    for i, h in enumerate((hA, hB)):
                        nc.tensor.matmul(
                            ctx_ps[i], lhsT=mm(v_sb[:, kt, h, :]),
                            rhs=mm(ex[:, i * QC:(i + 1) * QC]),
                            start=(kt == 0), stop=(kt == NT - 1))
                # normalize: cT = ctx[0:D] * (1/denom) broadcast over partitions.
                # Stage ctx psum to SBUF right away so the PSUM bank frees for
                # the next q-chunk; recip/replicate/mult run off-critical-path.
                for i, h in enumerate((hA, hB)):
                    ctxu = ctxu_pool.tile([D, QC], f32, tag="ctxu",
                                          name=f"ctxu{p}_{qc}_{i}")
                    nc.vector.tensor_copy(ctxu, ctx_ps[i][0:D, :])
                    # custom-DVE recip requires partition-0-based input: stage
                    # the denominator row to a base-0 tile first.
                    den = rc_pool.tile([1, QC], f32, tag="den")
                    nc.vector.tensor_copy(den, ctx_ps[i][D:D + 1, :])
                    rc = rc_pool.tile([1, QC], f32, tag="rc")
                    if cfg.mm_dtype == "bf16":
                        nc.vector.reciprocal_approx_fast(rc, den)
                    else:
                        rcs = rc_pool.tile([1, QC], f32, tag="rcs")
                        nc.vector.reciprocal_approx_accurate(rc, den, scratch=rcs)
                    # replicate 1/denom across 64 partitions on the (idle)
                    # GpSimd engine -- no PE or PSUM involvement
                    rep = rc_pool.tile([D, QC], f32, tag="rep")
                    nc.gpsimd.partition_broadcast(rep, rc)
                    nc.vector.tensor_tensor(
                        out=cT_sb[(h % 2) * D:(h % 2) * D + D, p, qs],
                        in0=ctxu[0:D, :], in1=rep, op=mult)
            if stop_after == "attn":
                continue
            # out-projection for this q-chunk's s-tiles (overlaps the next
            # q-chunk's exp work on ACT)
            for stl in range(QC // 128):
                st = qc * (QC // 128) + stl
                ss = slice(st * 128, (st + 1) * 128)
                ot = out_pool.tile([128, cfg.H], f32, tag="ot")
                for oc in range(NOC):
                    po = ps_ctx.tile([128, OC], f32, tag="ctx", name=f"po{st}_{oc}")
                    for ft in range(2):
                        nc.tensor.matmul(
                            po, lhsT=mm(cT_sb[:, ft, ss]),
                            rhs=mm(wo_sb[:, ft, oc * OC:(oc + 1) * OC]),
                            start=(ft == 0), stop=(ft == 1))
                    nc.vector.tensor_copy(ot[:, oc * OC:(oc + 1) * OC], po)
                nc.sync.dma_start(out=t["out"][ss, :], in_=ot)

        if "dbg_cT" in t:
            nc.sync.dma_start(out=t["dbg_cT"], in_=cT_sb)


def build(cfg: Cfg, dbg=False, stop_after=None):
    """Build + compile the per-core program. Returns (nc, input names)."""
    import concourse.tile as tile
    from concourse import bacc, mybir

    f32 = mybir.dt.float32
    dmm = {"bf16": mybir.dt.bfloat16, "fp32": f32, "fp32r": f32}[cfg.mm_dtype]

    nc = bacc.Bacc("TRN2", debug=False, enable_asserts=False,
                   target_bir_lowering=False)
    t = {}
    t["xT"] = nc.dram_tensor("xT", [cfg.H, cfg.S], dmm, kind="ExternalInput").ap()
    t["wqk"] = nc.dram_tensor("wqk", [cfg.H, 4 * 128], dmm, kind="ExternalInput").ap()
    t["wv"] = nc.dram_tensor("wv", [cfg.H, HPC * D], dmm, kind="ExternalInput").ap()
    t["wo"] = nc.dram_tensor("wo", [HPC * D, cfg.H], dmm, kind="ExternalInput").ap()
    t["cosT"] = nc.dram_tensor("cosT", [128, cfg.S], f32, kind="ExternalInput").ap()
    t["sinS"] = nc.dram_tensor("sinS", [128, cfg.S], f32, kind="ExternalInput").ap()
    t["mbias"] = nc.dram_tensor("mbias", [128, cfg.NT], f32, kind="ExternalInput").ap()
    t["out"] = nc.dram_tensor("out", [cfg.S, cfg.H], f32, kind="ExternalOutput").ap()
    if dbg:
        t["dbg_qk"] = nc.dram_tensor(
            "dbg_qk", [128, 4, cfg.S], dmm, kind="ExternalOutput").ap()
        t["dbg_v"] = nc.dram_tensor(
            "dbg_v", [128, cfg.NT, HPC, D + 1], dmm, kind="ExternalOutput").ap()
        t["dbg_cT"] = nc.dram_tensor(
            "dbg_cT", [128, 2, cfg.S], dmm, kind="ExternalOutput").ap()

    with tile.TileContext(nc) as tc:
        _emit(tc, t, cfg, stop_after=stop_after)
    nc.compile()
    return nc


# ----------------------------------------------------------------------------
# Host side: shard, run, gather
# ----------------------------------------------------------------------------

def rope_tables(S, dtype=np.float32):
    """cos/sin tables exactly as the reference builds them."""
    inv_freq = 1.0 / (10000.0 ** (np.arange(0, D, 2, dtype=np.float32) / D))
    tt = np.arange(S, dtype=np.float32)
    freqs = np.einsum("i,j->ij", tt, inv_freq)  # [S, D/2]
    emb = np.concatenate([freqs, freqs], axis=-1)  # [S, D]
    return np.cos(emb).astype(dtype), np.sin(emb).astype(dtype)


def device_rope_tables(S):
    """(cosT, sinSh) in the [128, S] partition layout the kernel expects.

    cosT: cos table transposed, stacked twice (two heads per 128 partitions).
    sinSh: sign-folded sin table (rotate_half sign), transposed, stacked, and
    32-block swapped so the rope multiply reads it at the SOURCE partition
    base (walrus requires equal input base partitions on DVE tensor ops).
    """
    cos, sin = rope_tables(S)
    cosT = np.ascontiguousarray(np.tile(cos.T, (2, 1)))  # [128, S]
    sinT = sin.T.copy()  # [D, S]
    sinT[:D // 2, :] *= -1.0
    sinS = np.tile(sinT, (2, 1))  # [128, S], signed
    perm = np.r_[32:64, 0:32, 96:128, 64:96]
    sinSh = np.ascontiguousarray(sinS[perm])
    return cosT.astype(np.float32), sinSh.astype(np.float32)


def make_in_maps(x, attention_mask, w_qkv, w_out, cfg: Cfg):
    """Build the 8 per-core input maps (numpy prep only)."""
    import ml_dtypes
    np_mm = {"bf16": np.dtype(ml_dtypes.bfloat16),
             "fp32": np.float32, "fp32r": np.float32}[cfg.mm_dtype]

    S, H = cfg.S, cfg.H
    cosT, sinS = device_rope_tables(S)

    in_maps = []
    for core in range(N_CORES):
        b, g = core // GROUPS, core % GROUPS
        heads = [g * HPC + j for j in range(HPC)]  # global head ids
        # Reference reshapes qkv to [B,S,16,192]: head h owns w_qkv columns
        # [h*3D, (h+1)*3D) as [q (D) | k (D) | v (D)].
        qcols = np.concatenate([np.arange(h * 3 * D, h * 3 * D + D) for h in heads])
        kcols = np.concatenate([np.arange(h * 3 * D + D, h * 3 * D + 2 * D) for h in heads])
        vcols = np.concatenate([np.arange(h * 3 * D + 2 * D, h * 3 * D + 3 * D) for h in heads])
        wqk = np.ascontiguousarray(
            np.concatenate([w_qkv[:, qcols], w_qkv[:, kcols]], axis=1)).astype(np_mm)
        wv = np.ascontiguousarray(w_qkv[:, vcols]).astype(np_mm)
        wo = np.ascontiguousarray(
            w_out[heads[0] * D:(heads[-1] + 1) * D, :]).astype(np_mm)
        xT = np.ascontiguousarray(np.asarray(x)[b].T).astype(np_mm)  # [H, S]
        mask = np.asarray(attention_mask)[b].astype(np.float32)  # [S]
        mbias = ((mask - 1.0) * 3e38).reshape(cfg.NT, 128).T.copy()  # [128, NT]
        in_maps.append({
            "xT": xT, "wqk": wqk, "wv": wv, "wo": wo,
            "cosT": cosT, "sinS": sinS, "mbias": mbias.astype(np.float32),
        })
    return in_maps


_PROG_CACHE = {}


def get_program(cfg: Cfg):
    if cfg not in _PROG_CACHE:
        _PROG_CACHE[cfg] = build(cfg)
    return _PROG_CACHE[cfg]


def run_sharded(x, attention_mask, w_qkv, w_out, trace=False, cfg: Cfg = None):
    """Run on the 8 NeuronCores; returns (full_output, BassKernelResults)."""
    from concourse import bass_utils

    cfg = cfg or Cfg()
    nc = get_program(cfg)
    in_maps = make_in_maps(x, attention_mask, w_qkv, w_out, cfg)
    res = bass_utils.run_bass_kernel_spmd(
        nc, in_maps, core_ids=list(range(N_CORES)), trace=trace)
    parts = [r["out"].astype(np.float64) for r in res.results]
    out = np.stack([
        sum(parts[b * GROUPS + g] for g in range(GROUPS))
        for b in range(B)
    ]).astype(np.float32)  # [B, S, H]
    return out, res


def kernel(x=None, attention_mask=None, w_qkv=None, w_out=None, **_ignored):
    out, _ = run_sharded(x, attention_mask, w_qkv, w_out, trace=False)
    return out


# revision 31
# speedup vs baseline: 1.2776x; 1.1584x over previous
"""Trainium2 Bass kernel: fused multi-head attention (QKV proj + RoPE +
softmax attention + output projection).

Problem dims: x[B=2, S=2048, H=1024], 16 heads, head_dim 64, fp32.

Sharding (8 NeuronCores): core = (batch b, head-group g); each core owns
batch b and 4 heads [4g..4g+4). It computes, fully on-device:
  - qkv projection for its heads (q/k produced feature-major, v seq-major)
  - RoPE on q/k
  - scoresT = k_rope^T-layout scores, exp (with mask bias + 1/sqrt(d) scale)
  - context via exp-scores @ v with an appended ones-column that yields the
    softmax denominators for free; per-query normalization
  - output projection against its 256 rows of w_out -> partial [S, 1024]
Host: shards/casts inputs per core, then sums the 4 per-batch partials.

The kernel is self-contained: call kernel(**inputs) with the full unsharded
inputs from setup_inputs(); returns the full [2, 2048, 1024] fp32 output.
"""

import math
import os
import sys
from dataclasses import dataclass

import numpy as np

for _p in ("/root/.axon_site/_ro/trn_rl_repo", "/opt/trn_rl_repo"):
    if _p not in sys.path and os.path.isdir(_p):
        sys.path.append(_p)

# Problem constants (hardcoded per spec; do not read spec.json at runtime).
B = 2
S_FULL = 2048
H_FULL = 1024
NUM_HEADS = 16
D = 64  # head dim
N_CORES = 8
GROUPS = N_CORES // B  # head groups per batch = 4
HPC = NUM_HEADS // GROUPS  # heads per core = 4 (2 pairs)

# Matmul operand dtype: "bf16" (fast), "fp32" (exact), "fp32r" (middle).
MM_DTYPE = os.environ.get("KERNEL_MM_DTYPE", "bf16")


@dataclass(frozen=True)
class Cfg:
    S: int = S_FULL
    H: int = H_FULL
    mm_dtype: str = MM_DTYPE

    @property
    def NT(self):  # 128-wide seq tiles (key tiles / s tiles)
        return self.S // 128

    @property
    def QC(self):  # query-chunk width (matmul N)
        return min(512, self.S)

    @property
    def NQC(self):
        return self.S // self.QC

    @property
    def HT(self):  # hidden contraction tiles
        return self.H // 128

    @property
    def OC(self):  # out-proj N chunk
        return min(512, self.H)

    @property
    def NOC(self):
        return self.H // self.OC


def _emit(tc, t, cfg, stop_after=None):
    """Emit the per-core program. `t` maps dram tensor name -> AP."""
    import concourse.bass as bass
    from concourse import mybir

    nc = tc.nc
    f32 = mybir.dt.float32
    dmm = {"bf16": mybir.dt.bfloat16, "fp32": f32, "fp32r": f32}[cfg.mm_dtype]

    if cfg.mm_dtype == "fp32r":
        mm = lambda ap: ap.bitcast(mybir.dt.float32r)
    else:
        mm = lambda ap: ap

    S, NT, QC, NQC, HT, OC, NOC = (
        cfg.S, cfg.NT, cfg.QC, cfg.NQC, cfg.HT, cfg.OC, cfg.NOC)
    Exp = mybir.ActivationFunctionType.Exp
    mult = mybir.AluOpType.mult
    add = mybir.AluOpType.add

    import contextlib
    es = contextlib.ExitStack()
    with es:
        consts = es.enter_context(tc.tile_pool(name="consts", bufs=1))
        xpool = es.enter_context(tc.tile_pool(name="xpool", bufs=2))
        store = es.enter_context(tc.tile_pool(name="store", bufs=1))
        rot_pool = es.enter_context(tc.tile_pool(name="rot", bufs=2))
        exp_pool = es.enter_context(tc.tile_pool(name="expp", bufs=6))
        rc_pool = es.enter_context(tc.tile_pool(name="rcp", bufs=3))
        ctxu_pool = es.enter_context(tc.tile_pool(name="ctxu", bufs=3))
        out_pool = es.enter_context(tc.tile_pool(name="outp", bufs=6))
        ps_big = es.enter_context(tc.tile_pool(name="ps_big", bufs=2, space="PSUM"))
        ps_ctx = es.enter_context(tc.tile_pool(name="ps_ctx", bufs=4, space="PSUM"))

        # ---- constants / weights to SBUF ----
        # Order matters: the first x-chunk + wqk unblock the first matmuls.
        wqk_sb = consts.tile([128, HT, 4 * 128], dmm)
        for ht in range(HT):
            nc.sync.dma_start(out=wqk_sb[:, ht, :], in_=t["wqk"][ht * 128:(ht + 1) * 128, :])
        x0 = xpool.tile([128, HT, QC], dmm, tag="xt", name="xt_sc0")
        for ht in range(HT):
            nc.sync.dma_start(out=x0[:, ht, :], in_=t["xT"][ht * 128:(ht + 1) * 128, 0:QC])
        cos_sb = consts.tile([128, S], f32)
        nc.sync.dma_start(out=cos_sb, in_=t["cosT"])
        sin_sb = consts.tile([128, S], f32)
        nc.sync.dma_start(out=sin_sb, in_=t["sinS"])
        mb_sb = consts.tile([128, NT], f32)
        nc.sync.dma_start(out=mb_sb, in_=t["mbias"])
        wv_sb = consts.tile([128, HT, HPC * D], dmm)
        for ht in range(HT):
            nc.sync.dma_start(out=wv_sb[:, ht, :], in_=t["wv"][ht * 128:(ht + 1) * 128, :])
        wo_sb = consts.tile([128, 2, cfg.H], dmm)
        for ft in range(2):
            nc.sync.dma_start(out=wo_sb[:, ft, :], in_=t["wo"][ft * 128:(ft + 1) * 128, :])

        # ---- persistent activations ----
        # qk_sb f-tiles: 0 = q pair0 (heads 0,1), 1 = q pair1 (heads 2,3),
        #                2 = k pair0,              3 = k pair1
        qk_sb = store.tile([128, 4, S], dmm)
        v_sb = store.tile([128, NT, HPC, D + 1], dmm)
        cT_sb = store.tile([128, 2, S], dmm)

        # ---- phase 1: qkv projection (+ rope, + v staging) ----
        for sc in range(NQC):
            qs = slice(sc * QC, (sc + 1) * QC)
            if sc == 0:
                xt = x0
            else:
                xt = xpool.tile([128, HT, QC], dmm, tag="xt", name=f"xt_sc{sc}")
                for ht in range(HT):
                    nc.sync.dma_start(out=xt[:, ht, :],
                                      in_=t["xT"][ht * 128:(ht + 1) * 128, qs])
            for f in range(4):
                ps = ps_ctx.tile([128, QC], f32, tag="ctx", name=f"qkps{sc}_{f}")
                for ht in range(HT):
                    nc.tensor.matmul(
                        ps, lhsT=mm(wqk_sb[:, ht, f * 128:(f + 1) * 128]),
                        rhs=mm(xt[:, ht, :]),
                        start=(ht == 0), stop=(ht == HT - 1))
                # Free the PSUM bank fast via an ACT copy (ACT is idle in this
                # phase), then run RoPE on SBUF where DVE gets 2x mode:
                #   qk = raw * cos + shift32(raw) * sin_signed
                # DVE requires equal input base partitions, so sin_sb holds a
                # 32-block-swapped table (sinSh[sq] == sin_signed[dq]) and the
                # quadrant crossing happens only on the output write.
                raw = rot_pool.tile([128, QC], f32, tag="raw")
                nc.scalar.copy(raw, ps)
                tr = rot_pool.tile([128, QC], f32, tag="tr")
                for dq, sq in ((0, 32), (32, 0), (64, 96), (96, 64)):
                    nc.vector.tensor_tensor(
                        out=tr[dq:dq + 32, :], in0=raw[sq:sq + 32, :],
                        in1=sin_sb[sq:sq + 32, qs], op=mult)
                qk_slice = qk_sb[:, f, qs]
                nc.vector.tensor_tensor(out=qk_slice, in0=raw, in1=cos_sb[:, qs], op=mult)
                nc.vector.tensor_tensor(out=qk_slice, in0=qk_slice, in1=tr, op=add)
            # v for the s-tiles inside this chunk (seq-major, + ones col)
            for stl in range(QC // 128):
                st = sc * (QC // 128) + stl
                psv = ps_ctx.tile([128, HPC * D], f32, tag="ctx")
                for ht in range(HT):
                    nc.tensor.matmul(
                        psv, lhsT=mm(xt[:, ht, stl * 128:(stl + 1) * 128]),
                        rhs=mm(wv_sb[:, ht, :]),
                        start=(ht == 0), stop=(ht == HT - 1))
                nc.vector.tensor_copy(
                    v_sb[:, st, :, 0:D],
                    psv.rearrange("p (h d) -> p h d", h=HPC))
                nc.vector.memset(v_sb[:, st, :, D:D + 1], 1.0)

        if "dbg_qk" in t:
            nc.sync.dma_start(out=t["dbg_qk"], in_=qk_sb)
            nc.sync.dma_start(out=t["dbg_v"], in_=v_sb)
        if stop_after == "qkv":
            return

        # ---- phase 2+3: attention with out-projection interleaved ----
        # Out-proj work for q-chunk qc is drip-fed into the PE stream during
        # q-chunk qc+1's attention (the PE has slack each kt iteration while
        # ACT chews on the exps), so ACT never starves at chunk boundaries.
        pending_out = []  # list of (st, oc, ot_tile)

        def emit_outproj_unit():
            if not pending_out:
                return
            st, oc, ot = pending_out.pop(0)
            ss = slice(st * 128, (st + 1) * 128)
            po = ps_ctx.tile([128, OC], f32, tag="ctx", name=f"po{st}_{oc}")
            for ft in range(2):
                nc.tensor.matmul(
                    po, lhsT=mm(cT_sb[:, ft, ss]),
                    rhs=mm(wo_sb[:, ft, oc * OC:(oc + 1) * OC]),
                    start=(ft == 0), stop=(ft == 1))
            nc.vector.tensor_copy(ot[:, oc * OC:(oc + 1) * OC], po)
            if oc == NOC - 1:
                nc.sync.dma_start(out=t["out"][ss, :], in_=ot)

        inv_sqrt_d = 1.0 / math.sqrt(D)
        for qc in range(NQC):
            qs = slice(qc * QC, (qc + 1) * QC)
            for p in range(2):
                hA, hB = 2 * p, 2 * p + 1
                ctx_ps = [ps_ctx.tile([D + 1, QC], f32, tag="ctx", name=f"ctxps{p}_{qc}_{i}")
                          for i in range(2)]
                for kt in range(NT):
                    ks = slice(kt * 128, (kt + 1) * 128)
                    sc_ps = ps_big.tile([128, 2 * QC], f32, tag="big")
                    for i, pb in enumerate((0, 64)):
                        nc.tensor.matmul(
                            sc_ps[:, i * QC:(i + 1) * QC],
                            lhsT=mm(qk_sb[pb:pb + 64, 2 + p, ks]),
                            rhs=mm(qk_sb[pb:pb + 64, p, qs]),
                            start=True, stop=True)
                    ex = exp_pool.tile([128, 2 * QC], dmm, tag="expT")
                    nc.scalar.activation(ex, sc_ps, Exp,
                                         bias=mb_sb[:, kt:kt + 1], scale=inv_sqrt_d)
                    for i, h in enumerate((hA, hB)):
                        nc.tensor.matmul(
                            ctx_ps[i], lhsT=mm(v_sb[:, kt, h, :]),
                            rhs=mm(ex[:, i * QC:(i + 1) * QC]),
                            start=(kt == 0), stop=(kt == NT - 1))
                    if kt % 4 == 3:
                        emit_outproj_unit()
                # normalize: cT = ctx[0:D] * (1/denom) broadcast over partitions.
                # Stage ctx psum to SBUF right away so the PSUM bank frees for
                # the next q-chunk; recip/replicate/mult run off-critical-path.
                for i, h in enumerate((hA, hB)):
                    ctxu = ctxu_pool.tile([D, QC], f32, tag="ctxu",
                                          name=f"ctxu{p}_{qc}_{i}")
                    nc.vector.tensor_copy(ctxu, ctx_ps[i][0:D, :])
                    # custom-DVE recip requires partition-0-based input: stage
                    # the denominator row to a base-0 tile first.
                    den = rc_pool.tile([1, QC], f32, tag="den")
                    nc.vector.tensor_copy(den, ctx_ps[i][D:D + 1, :])
                    rc = rc_pool.tile([1, QC], f32, tag="rc")
                    if cfg.mm_dtype == "bf16":
                        nc.vector.reciprocal_approx_fast(rc, den)
                    else:
                        rcs = rc_pool.tile([1, QC], f32, tag="rcs")
                        nc.vector.reciprocal_approx_accurate(rc, den, scratch=rcs)
                    # replicate 1/denom across 64 partitions on the (idle)
                    # GpSimd engine -- no PE or PSUM involvement
                    rep = rc_pool.tile([D, QC], f32, tag="rep")
                    nc.gpsimd.partition_broadcast(rep, rc)
                    nc.vector.tensor_tensor(
                        out=cT_sb[(h % 2) * D:(h % 2) * D + D, p, qs],
                        in0=ctxu[0:D, :], in1=rep, op=mult)
            if stop_after == "attn":
                continue
            # queue this q-chunk's out-projection units (emitted during the
            # next chunk's attention; drained at the end)
            for stl in range(QC // 128):
                st = qc * (QC // 128) + stl
                ot = out_pool.tile([128, cfg.H], f32, tag="ot",
                                   name=f"ot{st}")
                for oc in range(NOC):
                    pending_out.append((st, oc, ot))

        while pending_out:
            emit_outproj_unit()

        if "dbg_cT" in t:
            nc.sync.dma_start(out=t["dbg_cT"], in_=cT_sb)


def build(cfg: Cfg, dbg=False, stop_after=None):
    """Build + compile the per-core program. Returns (nc, input names)."""
    import concourse.tile as tile
    from concourse import bacc, mybir

    f32 = mybir.dt.float32
    dmm = {"bf16": mybir.dt.bfloat16, "fp32": f32, "fp32r": f32}[cfg.mm_dtype]

    nc = bacc.Bacc("TRN2", debug=False, enable_asserts=False,
                   target_bir_lowering=False)
    t = {}
    t["xT"] = nc.dram_tensor("xT", [cfg.H, cfg.S], dmm, kind="ExternalInput").ap()
    t["wqk"] = nc.dram_tensor("wqk", [cfg.H, 4 * 128], dmm, kind="ExternalInput").ap()
    t["wv"] = nc.dram_tensor("wv", [cfg.H, HPC * D], dmm, kind="ExternalInput").ap()
    t["wo"] = nc.dram_tensor("wo", [HPC * D, cfg.H], dmm, kind="ExternalInput").ap()
    t["cosT"] = nc.dram_tensor("cosT", [128, cfg.S], f32, kind="ExternalInput").ap()
    t["sinS"] = nc.dram_tensor("sinS", [128, cfg.S], f32, kind="ExternalInput").ap()
    t["mbias"] = nc.dram_tensor("mbias", [128, cfg.NT], f32, kind="ExternalInput").ap()
    t["out"] = nc.dram_tensor("out", [cfg.S, cfg.H], f32, kind="ExternalOutput").ap()
    if dbg:
        t["dbg_qk"] = nc.dram_tensor(
            "dbg_qk", [128, 4, cfg.S], dmm, kind="ExternalOutput").ap()
        t["dbg_v"] = nc.dram_tensor(
            "dbg_v", [128, cfg.NT, HPC, D + 1], dmm, kind="ExternalOutput").ap()
        t["dbg_cT"] = nc.dram_tensor(
            "dbg_cT", [128, 2, cfg.S], dmm, kind="ExternalOutput").ap()

    with tile.TileContext(nc) as tc:
        _emit(tc, t, cfg, stop_after=stop_after)
    nc.compile()
    return nc


# ----------------------------------------------------------------------------
# Host side: shard, run, gather
# ----------------------------------------------------------------------------

def rope_tables(S, dtype=np.float32):
    """cos/sin tables exactly as the reference builds them."""
    inv_freq = 1.0 / (10000.0 ** (np.arange(0, D, 2, dtype=np.float32) / D))
    tt = np.arange(S, dtype=np.float32)
    freqs = np.einsum("i,j->ij", tt, inv_freq)  # [S, D/2]
    emb = np.concatenate([freqs, freqs], axis=-1)  # [S, D]
    return np.cos(emb).astype(dtype), np.sin(emb).astype(dtype)


def device_rope_tables(S):
    """(cosT, sinSh) in the [128, S] partition layout the kernel expects.

    cosT: cos table transposed, stacked twice (two heads per 128 partitions).
    sinSh: sign-folded sin table (rotate_half sign), transposed, stacked, and
    32-block swapped so the rope multiply reads it at the SOURCE partition
    base (walrus requires equal input base partitions on DVE tensor ops).
    """
    cos, sin = rope_tables(S)
    cosT = np.ascontiguousarray(np.tile(cos.T, (2, 1)))  # [128, S]
    sinT = sin.T.copy()  # [D, S]
    sinT[:D // 2, :] *= -1.0
    sinS = np.tile(sinT, (2, 1))  # [128, S], signed
    perm = np.r_[32:64, 0:32, 96:128, 64:96]
    sinSh = np.ascontiguousarray(sinS[perm])
    return cosT.astype(np.float32), sinSh.astype(np.float32)


def make_in_maps(x, attention_mask, w_qkv, w_out, cfg: Cfg):
    """Build the 8 per-core input maps (numpy prep only)."""
    import ml_dtypes
    np_mm = {"bf16": np.dtype(ml_dtypes.bfloat16),
             "fp32": np.float32, "fp32r": np.float32}[cfg.mm_dtype]

    S, H = cfg.S, cfg.H
    cosT, sinS = device_rope_tables(S)

    in_maps = []
    for core in range(N_CORES):
        b, g = core // GROUPS, core % GROUPS
        heads = [g * HPC + j for j in range(HPC)]  # global head ids
        # Reference reshapes qkv to [B,S,16,192]: head h owns w_qkv columns
        # [h*3D, (h+1)*3D) as [q (D) | k (D) | v (D)].
        qcols = np.concatenate([np.arange(h * 3 * D, h * 3 * D + D) for h in heads])
        kcols = np.concatenate([np.arange(h * 3 * D + D, h * 3 * D + 2 * D) for h in heads])
        vcols = np.concatenate([np.arange(h * 3 * D + 2 * D, h * 3 * D + 3 * D) for h in heads])
        wqk = np.ascontiguousarray(
            np.concatenate([w_qkv[:, qcols], w_qkv[:, kcols]], axis=1)).astype(np_mm)
        wv = np.ascontiguousarray(w_qkv[:, vcols]).astype(np_mm)
        wo = np.ascontiguousarray(
            w_out[heads[0] * D:(heads[-1] + 1) * D, :]).astype(np_mm)
        xT = np.ascontiguousarray(np.asarray(x)[b].T).astype(np_mm)  # [H, S]
        mask = np.asarray(attention_mask)[b].astype(np.float32)  # [S]
        mbias = ((mask - 1.0) * 3e38).reshape(cfg.NT, 128).T.copy()  # [128, NT]
        in_maps.append({
            "xT": xT, "wqk": wqk, "wv": wv, "wo": wo,
            "cosT": cosT, "sinS": sinS, "mbias": mbias.astype(np.float32),
        })
    return in_maps


_PROG_CACHE = {}


def get_program(cfg: Cfg):
    if cfg not in _PROG_CACHE:
        _PROG_CACHE[cfg] = build(cfg)
    return _PROG_CACHE[cfg]


def run_sharded(x, attention_mask, w_qkv, w_out, trace=False, cfg: Cfg = None):
    """Run on the 8 NeuronCores; returns (full_output, BassKernelResults)."""
    from concourse import bass_utils

    cfg = cfg or Cfg()
    nc = get_program(cfg)
    in_maps = make_in_maps(x, attention_mask, w_qkv, w_out, cfg)
    res = bass_utils.run_bass_kernel_spmd(
        nc, in_maps, core_ids=list(range(N_CORES)), trace=trace)
    parts = [r["out"].astype(np.float64) for r in res.results]
    out = np.stack([
        sum(parts[b * GROUPS + g] for g in range(GROUPS))
        for b in range(B)
    ]).astype(np.float32)  # [B, S, H]
    return out, res


def kernel(x=None, attention_mask=None, w_qkv=None, w_out=None, **_ignored):
    out, _ = run_sharded(x, attention_mask, w_qkv, w_out, trace=False)
    return out


# revision 33
# speedup vs baseline: 1.3645x; 1.0680x over previous
"""Trainium2 Bass kernel: fused multi-head attention (QKV proj + RoPE +
softmax attention + output projection).

Problem dims: x[B=2, S=2048, H=1024], 16 heads, head_dim 64, fp32.

Sharding (8 NeuronCores): core = (batch b, head-group g); each core owns
batch b and 4 heads [4g..4g+4). It computes, fully on-device:
  - qkv projection for its heads (q/k produced feature-major, v seq-major)
  - RoPE on q/k
  - scoresT = k_rope^T-layout scores, exp (with mask bias + 1/sqrt(d) scale)
  - context via exp-scores @ v with an appended ones-column that yields the
    softmax denominators for free; per-query normalization
  - output projection against its 256 rows of w_out -> partial [S, 1024]
Host: shards/casts inputs per core, then sums the 4 per-batch partials.

The kernel is self-contained: call kernel(**inputs) with the full unsharded
inputs from setup_inputs(); returns the full [2, 2048, 1024] fp32 output.
"""

import math
import os
import sys
from dataclasses import dataclass

import numpy as np

for _p in ("/root/.axon_site/_ro/trn_rl_repo", "/opt/trn_rl_repo"):
    if _p not in sys.path and os.path.isdir(_p):
        sys.path.append(_p)

# Problem constants (hardcoded per spec; do not read spec.json at runtime).
B = 2
S_FULL = 2048
H_FULL = 1024
NUM_HEADS = 16
D = 64  # head dim
N_CORES = 8
GROUPS = N_CORES // B  # head groups per batch = 4
HPC = NUM_HEADS // GROUPS  # heads per core = 4 (2 pairs)

# Matmul operand dtype: "bf16" (fast), "fp32" (exact), "fp32r" (middle).
MM_DTYPE = os.environ.get("KERNEL_MM_DTYPE", "bf16")


@dataclass(frozen=True)
class Cfg:
    S: int = S_FULL
    H: int = H_FULL
    mm_dtype: str = MM_DTYPE

    @property
    def NT(self):  # 128-wide seq tiles (key tiles / s tiles)
        return self.S // 128

    @property
    def QC(self):  # query-chunk width (matmul N)
        return min(512, self.S)

    @property
    def NQC(self):
        return self.S // self.QC

    @property
    def HT(self):  # hidden contraction tiles
        return self.H // 128

    @property
    def OC(self):  # out-proj N chunk
        return min(512, self.H)

    @property
    def NOC(self):
        return self.H // self.OC


def _emit(tc, t, cfg, stop_after=None):
    """Emit the per-core program. `t` maps dram tensor name -> AP."""
    import concourse.bass as bass
    from concourse import mybir

    nc = tc.nc
    f32 = mybir.dt.float32
    dmm = {"bf16": mybir.dt.bfloat16, "fp32": f32, "fp32r": f32}[cfg.mm_dtype]

    if cfg.mm_dtype == "fp32r":
        mm = lambda ap: ap.bitcast(mybir.dt.float32r)
    else:
        mm = lambda ap: ap

    S, NT, QC, NQC, HT, OC, NOC = (
        cfg.S, cfg.NT, cfg.QC, cfg.NQC, cfg.HT, cfg.OC, cfg.NOC)
    Exp = mybir.ActivationFunctionType.Exp
    mult = mybir.AluOpType.mult
    add = mybir.AluOpType.add

    import contextlib
    es = contextlib.ExitStack()
    with es:
        consts = es.enter_context(tc.tile_pool(name="consts", bufs=1))
        xpool = es.enter_context(tc.tile_pool(name="xpool", bufs=2))
        store = es.enter_context(tc.tile_pool(name="store", bufs=1))
        rot_pool = es.enter_context(tc.tile_pool(name="rot", bufs=2))
        exp_pool = es.enter_context(tc.tile_pool(name="expp", bufs=6))
        rc_pool = es.enter_context(tc.tile_pool(name="rcp", bufs=3))
        ctxu_pool = es.enter_context(tc.tile_pool(name="ctxu", bufs=3))
        out_pool = es.enter_context(tc.tile_pool(name="outp", bufs=6))
        ps_big = es.enter_context(tc.tile_pool(name="ps_big", bufs=2, space="PSUM"))
        ps_ctx = es.enter_context(tc.tile_pool(name="ps_ctx", bufs=4, space="PSUM"))

        # ---- constants / weights to SBUF ----
        # Order matters: the first x-chunk + wqk unblock the first matmuls.
        wqk_sb = consts.tile([128, HT, 4 * 128], dmm)
        for ht in range(HT):
            nc.sync.dma_start(out=wqk_sb[:, ht, :], in_=t["wqk"][ht * 128:(ht + 1) * 128, :])
        x0 = xpool.tile([128, HT, QC], dmm, tag="xt", name="xt_sc0")
        for ht in range(HT):
            nc.sync.dma_start(out=x0[:, ht, :], in_=t["xT"][ht * 128:(ht + 1) * 128, 0:QC])
        cos_sb = consts.tile([128, S], f32)
        nc.sync.dma_start(out=cos_sb, in_=t["cosT"])
        sin_sb = consts.tile([128, S], f32)
        nc.sync.dma_start(out=sin_sb, in_=t["sinS"])
        mb_sb = consts.tile([128, NT], f32)
        nc.sync.dma_start(out=mb_sb, in_=t["mbias"])
        wv_sb = consts.tile([128, HT, HPC * D], dmm)
        for ht in range(HT):
            nc.sync.dma_start(out=wv_sb[:, ht, :], in_=t["wv"][ht * 128:(ht + 1) * 128, :])
        wo_sb = consts.tile([128, 2, cfg.H], dmm)
        for ft in range(2):
            nc.sync.dma_start(out=wo_sb[:, ft, :], in_=t["wo"][ft * 128:(ft + 1) * 128, :])

        # ---- persistent activations ----
        # qk_sb f-tiles: 0 = q pair0 (heads 0,1), 1 = q pair1 (heads 2,3),
        #                2 = k pair0,              3 = k pair1
        qk_sb = store.tile([128, 4, S], dmm)
        v_sb = store.tile([128, NT, HPC, D + 1], dmm)
        cT_sb = store.tile([128, 2, S], dmm)

        # ---- phase 1: qkv projection (+ rope, + v staging) ----
        for sc in range(NQC):
            qs = slice(sc * QC, (sc + 1) * QC)
            if sc == 0:
                xt = x0
            else:
                xt = xpool.tile([128, HT, QC], dmm, tag="xt", name=f"xt_sc{sc}")
                for ht in range(HT):
                    nc.sync.dma_start(out=xt[:, ht, :],
                                      in_=t["xT"][ht * 128:(ht + 1) * 128, qs])
            for f in range(4):
                ps = ps_ctx.tile([128, QC], f32, tag="ctx", name=f"qkps{sc}_{f}")
                for ht in range(HT):
                    nc.tensor.matmul(
                        ps, lhsT=mm(wqk_sb[:, ht, f * 128:(f + 1) * 128]),
                        rhs=mm(xt[:, ht, :]),
                        start=(ht == 0), stop=(ht == HT - 1))
                # Free the PSUM bank fast via an ACT copy (ACT is idle in this
                # phase). The rotate-half partition shift is done by DMA
                # engines (4 small SBUF->SBUF copies), so every DVE op is
                # full-width:  qk = raw * cos + shifted(raw) * sin_signed
                raw = rot_pool.tile([128, QC], f32, tag="raw")
                nc.scalar.copy(raw, ps)
                shf = rot_pool.tile([128, QC], f32, tag="shf")
                for dq, sq in ((0, 32), (32, 0), (64, 96), (96, 64)):
                    nc.sync.dma_start(out=shf[dq:dq + 32, :],
                                      in_=raw[sq:sq + 32, :])
                tr = rot_pool.tile([128, QC], f32, tag="tr")
                nc.vector.tensor_tensor(out=tr, in0=shf, in1=sin_sb[:, qs], op=mult)
                qk_slice = qk_sb[:, f, qs]
                nc.vector.tensor_tensor(out=qk_slice, in0=raw, in1=cos_sb[:, qs], op=mult)
                nc.vector.tensor_tensor(out=qk_slice, in0=qk_slice, in1=tr, op=add)
            # v for the s-tiles inside this chunk (seq-major, + ones col)
            for stl in range(QC // 128):
                st = sc * (QC // 128) + stl
                psv = ps_ctx.tile([128, HPC * D], f32, tag="ctx")
                for ht in range(HT):
                    nc.tensor.matmul(
                        psv, lhsT=mm(xt[:, ht, stl * 128:(stl + 1) * 128]),
                        rhs=mm(wv_sb[:, ht, :]),
                        start=(ht == 0), stop=(ht == HT - 1))
                nc.vector.tensor_copy(
                    v_sb[:, st, :, 0:D],
                    psv.rearrange("p (h d) -> p h d", h=HPC))
                nc.vector.memset(v_sb[:, st, :, D:D + 1], 1.0)

        if "dbg_qk" in t:
            nc.sync.dma_start(out=t["dbg_qk"], in_=qk_sb)
            nc.sync.dma_start(out=t["dbg_v"], in_=v_sb)
        if stop_after == "qkv":
            return

        # ---- phase 2+3: attention with out-projection interleaved ----
        # Out-proj work for q-chunk qc is drip-fed into the PE stream during
        # q-chunk qc+1's attention (the PE has slack each kt iteration while
        # ACT chews on the exps), so ACT never starves at chunk boundaries.
        pending_out = []  # list of (st, oc, ot_tile)

        def emit_outproj_unit():
            if not pending_out:
                return
            st, oc, ot = pending_out.pop(0)
            ss = slice(st * 128, (st + 1) * 128)
            po = ps_ctx.tile([128, OC], f32, tag="ctx", name=f"po{st}_{oc}")
            for ft in range(2):
                nc.tensor.matmul(
                    po, lhsT=mm(cT_sb[:, ft, ss]),
                    rhs=mm(wo_sb[:, ft, oc * OC:(oc + 1) * OC]),
                    start=(ft == 0), stop=(ft == 1))
            nc.vector.tensor_copy(ot[:, oc * OC:(oc + 1) * OC], po)
            if oc == NOC - 1:
                nc.sync.dma_start(out=t["out"][ss, :], in_=ot)

        inv_sqrt_d = 1.0 / math.sqrt(D)
        for qc in range(NQC):
            qs = slice(qc * QC, (qc + 1) * QC)
            for p in range(2):
                hA, hB = 2 * p, 2 * p + 1
                ctx_ps = [ps_ctx.tile([D + 1, QC], f32, tag="ctx", name=f"ctxps{p}_{qc}_{i}")
                          for i in range(2)]
                for kt in range(NT):
                    ks = slice(kt * 128, (kt + 1) * 128)
                    sc_ps = ps_big.tile([128, 2 * QC], f32, tag="big")
                    for i, pb in enumerate((0, 64)):
                        nc.tensor.matmul(
                            sc_ps[:, i * QC:(i + 1) * QC],
                            lhsT=mm(qk_sb[pb:pb + 64, 2 + p, ks]),
                            rhs=mm(qk_sb[pb:pb + 64, p, qs]),
                            start=True, stop=True)
                    ex = exp_pool.tile([128, 2 * QC], dmm, tag="expT")
                    nc.scalar.activation(ex, sc_ps, Exp,
                                         bias=mb_sb[:, kt:kt + 1], scale=inv_sqrt_d)
                    for i, h in enumerate((hA, hB)):
                        nc.tensor.matmul(
                            ctx_ps[i], lhsT=mm(v_sb[:, kt, h, :]),
                            rhs=mm(ex[:, i * QC:(i + 1) * QC]),
                            start=(kt == 0), stop=(kt == NT - 1))
                    if kt % 4 == 3:
                        emit_outproj_unit()
                # normalize: cT = ctx[0:D] * (1/denom) broadcast over partitions.
                # Stage ctx psum to SBUF right away so the PSUM bank frees for
                # the next q-chunk; recip/replicate/mult run off-critical-path.
                for i, h in enumerate((hA, hB)):
                    ctxu = ctxu_pool.tile([D, QC], f32, tag="ctxu",
                                          name=f"ctxu{p}_{qc}_{i}")
                    nc.vector.tensor_copy(ctxu, ctx_ps[i][0:D, :])
                    # custom-DVE recip requires partition-0-based input: stage
                    # the denominator row to a base-0 tile first.
                    den = rc_pool.tile([1, QC], f32, tag="den")
                    nc.vector.tensor_copy(den, ctx_ps[i][D:D + 1, :])
                    rc = rc_pool.tile([1, QC], f32, tag="rc")
                    if cfg.mm_dtype == "bf16":
                        nc.vector.reciprocal_approx_fast(rc, den)
                    else:
                        rcs = rc_pool.tile([1, QC], f32, tag="rcs")
                        nc.vector.reciprocal_approx_accurate(rc, den, scratch=rcs)
                    # replicate 1/denom across 64 partitions on the (idle)
                    # GpSimd engine -- no PE or PSUM involvement
                    rep = rc_pool.tile([D, QC], f32, tag="rep")
                    nc.gpsimd.partition_broadcast(rep, rc)
                    nc.vector.tensor_tensor(
                        out=cT_sb[(h % 2) * D:(h % 2) * D + D, p, qs],
                        in0=ctxu[0:D, :], in1=rep, op=mult)
            if stop_after == "attn":
                continue
            # queue this q-chunk's out-projection units (emitted during the
            # next chunk's attention; drained at the end)
            for stl in range(QC // 128):
                st = qc * (QC // 128) + stl
                ot = out_pool.tile([128, cfg.H], f32, tag="ot",
                                   name=f"ot{st}")
                for oc in range(NOC):
                    pending_out.append((st, oc, ot))

        while pending_out:
            emit_outproj_unit()

        if "dbg_cT" in t:
            nc.sync.dma_start(out=t["dbg_cT"], in_=cT_sb)


def build(cfg: Cfg, dbg=False, stop_after=None):
    """Build + compile the per-core program. Returns (nc, input names)."""
    import concourse.tile as tile
    from concourse import bacc, mybir

    f32 = mybir.dt.float32
    dmm = {"bf16": mybir.dt.bfloat16, "fp32": f32, "fp32r": f32}[cfg.mm_dtype]

    nc = bacc.Bacc("TRN2", debug=False, enable_asserts=False,
                   target_bir_lowering=False)
    t = {}
    t["xT"] = nc.dram_tensor("xT", [cfg.H, cfg.S], dmm, kind="ExternalInput").ap()
    t["wqk"] = nc.dram_tensor("wqk", [cfg.H, 4 * 128], dmm, kind="ExternalInput").ap()
    t["wv"] = nc.dram_tensor("wv", [cfg.H, HPC * D], dmm, kind="ExternalInput").ap()
    t["wo"] = nc.dram_tensor("wo", [HPC * D, cfg.H], dmm, kind="ExternalInput").ap()
    t["cosT"] = nc.dram_tensor("cosT", [128, cfg.S], f32, kind="ExternalInput").ap()
    t["sinS"] = nc.dram_tensor("sinS", [128, cfg.S], f32, kind="ExternalInput").ap()
    t["mbias"] = nc.dram_tensor("mbias", [128, cfg.NT], f32, kind="ExternalInput").ap()
    t["out"] = nc.dram_tensor("out", [cfg.S, cfg.H], f32, kind="ExternalOutput").ap()
    if dbg:
        t["dbg_qk"] = nc.dram_tensor(
            "dbg_qk", [128, 4, cfg.S], dmm, kind="ExternalOutput").ap()
        t["dbg_v"] = nc.dram_tensor(
            "dbg_v", [128, cfg.NT, HPC, D + 1], dmm, kind="ExternalOutput").ap()
        t["dbg_cT"] = nc.dram_tensor(
            "dbg_cT", [128, 2, cfg.S], dmm, kind="ExternalOutput").ap()

    with tile.TileContext(nc) as tc:
        _emit(tc, t, cfg, stop_after=stop_after)
    nc.compile()
    return nc


# ----------------------------------------------------------------------------
# Host side: shard, run, gather
# ----------------------------------------------------------------------------

def rope_tables(S, dtype=np.float32):
    """cos/sin tables exactly as the reference builds them."""
    inv_freq = 1.0 / (10000.0 ** (np.arange(0, D, 2, dtype=np.float32) / D))
    tt = np.arange(S, dtype=np.float32)
    freqs = np.einsum("i,j->ij", tt, inv_freq)  # [S, D/2]
    emb = np.concatenate([freqs, freqs], axis=-1)  # [S, D]
    return np.cos(emb).astype(dtype), np.sin(emb).astype(dtype)


def device_rope_tables(S):
    """(cosT, sinS) in the [128, S] partition layout the kernel expects.

    cosT: cos table transposed, stacked twice (two heads per 128 partitions).
    sinS: sign-folded sin table (rotate_half sign folded in), transposed,
    stacked twice.
    """
    cos, sin = rope_tables(S)
    cosT = np.ascontiguousarray(np.tile(cos.T, (2, 1)))  # [128, S]
    sinT = sin.T.copy()  # [D, S]
    sinT[:D // 2, :] *= -1.0
    sinS = np.ascontiguousarray(np.tile(sinT, (2, 1)))  # [128, S], signed
    return cosT.astype(np.float32), sinS.astype(np.float32)


def make_in_maps(x, attention_mask, w_qkv, w_out, cfg: Cfg):
    """Build the 8 per-core input maps (numpy prep only)."""
    import ml_dtypes
    np_mm = {"bf16": np.dtype(ml_dtypes.bfloat16),
             "fp32": np.float32, "fp32r": np.float32}[cfg.mm_dtype]

    S, H = cfg.S, cfg.H
    cosT, sinS = device_rope_tables(S)

    in_maps = []
    for core in range(N_CORES):
        b, g = core // GROUPS, core % GROUPS
        heads = [g * HPC + j for j in range(HPC)]  # global head ids
        # Reference reshapes qkv to [B,S,16,192]: head h owns w_qkv columns
        # [h*3D, (h+1)*3D) as [q (D) | k (D) | v (D)].
        qcols = np.concatenate([np.arange(h * 3 * D, h * 3 * D + D) for h in heads])
        kcols = np.concatenate([np.arange(h * 3 * D + D, h * 3 * D + 2 * D) for h in heads])
        vcols = np.concatenate([np.arange(h * 3 * D + 2 * D, h * 3 * D + 3 * D) for h in heads])
        wqk = np.ascontiguousarray(
            np.concatenate([w_qkv[:, qcols], w_qkv[:, kcols]], axis=1)).astype(np_mm)
        wv = np.ascontiguousarray(w_qkv[:, vcols]).astype(np_mm)
        wo = np.ascontiguousarray(
            w_out[heads[0] * D:(heads[-1] + 1) * D, :]).astype(np_mm)
        xT = np.ascontiguousarray(np.asarray(x)[b].T).astype(np_mm)  # [H, S]
        mask = np.asarray(attention_mask)[b].astype(np.float32)  # [S]
        mbias = ((mask - 1.0) * 3e38).reshape(cfg.NT, 128).T.copy()  # [128, NT]
        in_maps.append({
            "xT": xT, "wqk": wqk, "wv": wv, "wo": wo,
            "cosT": cosT, "sinS": sinS, "mbias": mbias.astype(np.float32),
        })
    return in_maps


_PROG_CACHE = {}


def get_program(cfg: Cfg):
    if cfg not in _PROG_CACHE:
        _PROG_CACHE[cfg] = build(cfg)
    return _PROG_CACHE[cfg]


def run_sharded(x, attention_mask, w_qkv, w_out, trace=False, cfg: Cfg = None):
    """Run on the 8 NeuronCores; returns (full_output, BassKernelResults)."""
    from concourse import bass_utils

    cfg = cfg or Cfg()
    nc = get_program(cfg)
    in_maps = make_in_maps(x, attention_mask, w_qkv, w_out, cfg)
    res = bass_utils.run_bass_kernel_spmd(
        nc, in_maps, core_ids=list(range(N_CORES)), trace=trace)
    parts = [r["out"].astype(np.float64) for r in res.results]
    out = np.stack([
        sum(parts[b * GROUPS + g] for g in range(GROUPS))
        for b in range(B)
    ]).astype(np.float32)  # [B, S, H]
    return out, res


def kernel(x=None, attention_mask=None, w_qkv=None, w_out=None, **_ignored):
    out, _ = run_sharded(x, attention_mask, w_qkv, w_out, trace=False)
    return out


# revision 36
# speedup vs baseline: 1.4039x; 1.0288x over previous
"""Trainium2 Bass kernel: fused multi-head attention (QKV proj + RoPE +
softmax attention + output projection).

Problem dims: x[B=2, S=2048, H=1024], 16 heads, head_dim 64, fp32.

Sharding (8 NeuronCores): core = (batch b, head-group g); each core owns
batch b and 4 heads [4g..4g+4). It computes, fully on-device:
  - qkv projection for its heads (q/k produced feature-major, v seq-major)
  - RoPE on q/k
  - scoresT = k_rope^T-layout scores, exp (with mask bias + 1/sqrt(d) scale)
  - context via exp-scores @ v with an appended ones-column that yields the
    softmax denominators for free; per-query normalization
  - output projection against its 256 rows of w_out -> partial [S, 1024]
Host: shards/casts inputs per core, then sums the 4 per-batch partials.

The kernel is self-contained: call kernel(**inputs) with the full unsharded
inputs from setup_inputs(); returns the full [2, 2048, 1024] fp32 output.
"""

import math
import os
import sys
from dataclasses import dataclass

import numpy as np

for _p in ("/root/.axon_site/_ro/trn_rl_repo", "/opt/trn_rl_repo"):
    if _p not in sys.path and os.path.isdir(_p):
        sys.path.append(_p)

# Problem constants (hardcoded per spec; do not read spec.json at runtime).
B = 2
S_FULL = 2048
H_FULL = 1024
NUM_HEADS = 16
D = 64  # head dim
N_CORES = 8
GROUPS = N_CORES // B  # head groups per batch = 4
HPC = NUM_HEADS // GROUPS  # heads per core = 4 (2 pairs)

# Matmul operand dtype: "bf16" (fast), "fp32" (exact), "fp32r" (middle).
MM_DTYPE = os.environ.get("KERNEL_MM_DTYPE", "bf16")


@dataclass(frozen=True)
class Cfg:
    S: int = S_FULL
    H: int = H_FULL
    mm_dtype: str = MM_DTYPE

    @property
    def NT(self):  # 128-wide seq tiles (key tiles / s tiles)
        return self.S // 128

    @property
    def QC(self):  # query-chunk width (matmul N)
        return min(512, self.S)

    @property
    def NQC(self):
        return self.S // self.QC

    @property
    def HT(self):  # hidden contraction tiles
        return self.H // 128

    @property
    def OC(self):  # out-proj N chunk
        return min(512, self.H)

    @property
    def NOC(self):
        return self.H // self.OC


def _emit(tc, t, cfg, stop_after=None):
    """Emit the per-core program. `t` maps dram tensor name -> AP."""
    import concourse.bass as bass
    from concourse import mybir

    nc = tc.nc
    f32 = mybir.dt.float32
    dmm = {"bf16": mybir.dt.bfloat16, "fp32": f32, "fp32r": f32}[cfg.mm_dtype]

    if cfg.mm_dtype == "fp32r":
        mm = lambda ap: ap.bitcast(mybir.dt.float32r)
    else:
        mm = lambda ap: ap

    S, NT, QC, NQC, HT, OC, NOC = (
        cfg.S, cfg.NT, cfg.QC, cfg.NQC, cfg.HT, cfg.OC, cfg.NOC)
    Exp = mybir.ActivationFunctionType.Exp
    mult = mybir.AluOpType.mult
    add = mybir.AluOpType.add

    import contextlib
    es = contextlib.ExitStack()
    with es:
        consts = es.enter_context(tc.tile_pool(name="consts", bufs=1))
        xpool = es.enter_context(tc.tile_pool(name="xpool", bufs=2))
        store = es.enter_context(tc.tile_pool(name="store", bufs=1))
        rot_pool = es.enter_context(tc.tile_pool(name="rot", bufs=2))
        exp_pool = es.enter_context(tc.tile_pool(name="expp", bufs=6))
        rc_pool = es.enter_context(tc.tile_pool(name="rcp", bufs=3))
        ctxu_pool = es.enter_context(tc.tile_pool(name="ctxu", bufs=3))
        out_pool = es.enter_context(tc.tile_pool(name="outp", bufs=6))
        ps_big = es.enter_context(tc.tile_pool(name="ps_big", bufs=2, space="PSUM"))
        ps_ctx = es.enter_context(tc.tile_pool(name="ps_ctx", bufs=4, space="PSUM"))

        # ---- constants / weights to SBUF ----
        # Order matters: the first x-chunk + wqk unblock the first matmuls.
        wqk_sb = consts.tile([128, HT, 4 * 128], dmm)
        for ht in range(HT):
            nc.sync.dma_start(out=wqk_sb[:, ht, :], in_=t["wqk"][ht * 128:(ht + 1) * 128, :])
        x0 = xpool.tile([128, HT, QC], dmm, tag="xt", name="xt_sc0")
        for ht in range(HT):
            nc.sync.dma_start(out=x0[:, ht, :], in_=t["xT"][ht * 128:(ht + 1) * 128, 0:QC])
        cos_sb = consts.tile([128, S], f32)
        nc.sync.dma_start(out=cos_sb, in_=t["cosT"])
        sin_sb = consts.tile([128, S], f32)
        nc.sync.dma_start(out=sin_sb, in_=t["sinS"])
        mb_sb = consts.tile([128, NT], f32)
        nc.sync.dma_start(out=mb_sb, in_=t["mbias"])
        wv_sb = consts.tile([128, HT, HPC * D], dmm)
        for ht in range(HT):
            nc.sync.dma_start(out=wv_sb[:, ht, :], in_=t["wv"][ht * 128:(ht + 1) * 128, :])
        wo_sb = consts.tile([128, 2, cfg.H], dmm)
        for ft in range(2):
            nc.sync.dma_start(out=wo_sb[:, ft, :], in_=t["wo"][ft * 128:(ft + 1) * 128, :])

        # ---- persistent activations ----
        # qk_sb f-tiles: 0 = q pair0 (heads 0,1), 1 = q pair1 (heads 2,3),
        #                2 = k pair0,              3 = k pair1
        qk_sb = store.tile([128, 4, S], dmm)
        v_sb = store.tile([128, NT, HPC, D + 1], dmm)
        cT_sb = store.tile([128, 2, S], dmm)

        inv_sqrt_d = 1.0 / math.sqrt(D)
        pending_out = []  # list of (st, oc, ot_tile)

        def emit_outproj_unit():
            if not pending_out:
                return
            st, oc, ot = pending_out.pop(0)
            ss = slice(st * 128, (st + 1) * 128)
            po = ps_ctx.tile([128, OC], f32, tag="ctx", name=f"po{st}_{oc}")
            for ft in range(2):
                nc.tensor.matmul(
                    po, lhsT=mm(cT_sb[:, ft, ss]),
                    rhs=mm(wo_sb[:, ft, oc * OC:(oc + 1) * OC]),
                    start=(ft == 0), stop=(ft == 1))
            nc.vector.tensor_copy(ot[:, oc * OC:(oc + 1) * OC], po)
            if oc == NOC - 1:
                nc.sync.dma_start(out=t["out"][ss, :], in_=ot)

        def queue_outproj(qc):
            for stl in range(QC // 128):
                st = qc * (QC // 128) + stl
                ot = out_pool.tile([128, cfg.H], f32, tag="ot", name=f"ot{st}")
                for oc in range(NOC):
                    pending_out.append((st, oc, ot))

        def attn_iteration(p, qc, kt, ctx_ps, drip=True):
            """scores (packed pair) -> exp -> 2 ctx-accumulate matmuls."""
            qs = slice(qc * QC, (qc + 1) * QC)
            ks = slice(kt * 128, (kt + 1) * 128)
            sc_ps = ps_big.tile([128, 2 * QC], f32, tag="big",
                                name=f"scps{p}_{qc}_{kt}")
            for i, pb in enumerate((0, 64)):
                nc.tensor.matmul(
                    sc_ps[:, i * QC:(i + 1) * QC],
                    lhsT=mm(qk_sb[pb:pb + 64, 2 + p, ks]),
                    rhs=mm(qk_sb[pb:pb + 64, p, qs]),
                    start=True, stop=True)
            ex = exp_pool.tile([128, 2 * QC], dmm, tag="expT",
                               name=f"ex{p}_{qc}_{kt}")
            nc.scalar.activation(ex, sc_ps, Exp,
                                 bias=mb_sb[:, kt:kt + 1], scale=inv_sqrt_d)
            for i, h in enumerate((2 * p, 2 * p + 1)):
                nc.tensor.matmul(
                    ctx_ps[i], lhsT=mm(v_sb[:, kt, h, :]),
                    rhs=mm(ex[:, i * QC:(i + 1) * QC]),
                    start=(kt == 0), stop=(kt == NT - 1))
            if drip and kt % 4 == 3:
                emit_outproj_unit()

        def normalize(p, qc, ctx_ps):
            """cT = ctx[0:D] * (1/denom) broadcast over partitions. Stage ctx
            psum to SBUF right away so the bank frees; the rest runs off the
            critical path."""
            qs = slice(qc * QC, (qc + 1) * QC)
            for i, h in enumerate((2 * p, 2 * p + 1)):
                ctxu = ctxu_pool.tile([D, QC], f32, tag="ctxu",
                                      name=f"ctxu{p}_{qc}_{i}")
                nc.vector.tensor_copy(ctxu, ctx_ps[i][0:D, :])
                # custom-DVE recip requires partition-0-based input: stage
                # the denominator row to a base-0 tile first.
                den = rc_pool.tile([1, QC], f32, tag="den")
                nc.vector.tensor_copy(den, ctx_ps[i][D:D + 1, :])
                rc = rc_pool.tile([1, QC], f32, tag="rc")
                if cfg.mm_dtype == "bf16":
                    nc.vector.reciprocal_approx_fast(rc, den)
                else:
                    rcs = rc_pool.tile([1, QC], f32, tag="rcs")
                    nc.vector.reciprocal_approx_accurate(rc, den, scratch=rcs)
                # replicate 1/denom across 64 partitions on the (idle)
                # GpSimd engine -- no PE or PSUM involvement
                rep = rc_pool.tile([D, QC], f32, tag="rep")
                nc.gpsimd.partition_broadcast(rep, rc)
                nc.vector.tensor_tensor(
                    out=cT_sb[(h % 2) * D:(h % 2) * D + D, p, qs],
                    in0=ctxu[0:D, :], in1=rep, op=mult)

        # ---- phase 1: qkv projection (+ rope, + v staging) ----
        # The first head-pair's qc0 attention is interleaved: its kt
        # iterations only need the s-chunks produced so far, and they keep
        # ACT (exp) and the PE busy through phase 1's dependency stalls.
        ctx00 = [ps_ctx.tile([D + 1, QC], f32, tag="ctx", name=f"ctxps0_0_{i}")
                 for i in range(2)]
        for sc in range(NQC):
            qs = slice(sc * QC, (sc + 1) * QC)
            if sc == 0:
                xt = x0
            else:
                xt = xpool.tile([128, HT, QC], dmm, tag="xt", name=f"xt_sc{sc}")
                for ht in range(HT):
                    nc.sync.dma_start(out=xt[:, ht, :],
                                      in_=t["xT"][ht * 128:(ht + 1) * 128, qs])
            for f in (0, 2, 1, 3):
                ps = ps_ctx.tile([128, QC], f32, tag="ctx", name=f"qkps{sc}_{f}")
                for ht in range(HT):
                    nc.tensor.matmul(
                        ps, lhsT=mm(wqk_sb[:, ht, f * 128:(f + 1) * 128]),
                        rhs=mm(xt[:, ht, :]),
                        start=(ht == 0), stop=(ht == HT - 1))
                # Free the PSUM bank fast via an ACT copy (ACT is idle in this
                # phase). The rotate-half partition shift is done by DMA
                # engines (4 small SBUF->SBUF copies), so every DVE op is
                # full-width:  qk = raw * cos + shifted(raw) * sin_signed
                raw = rot_pool.tile([128, QC], f32, tag="raw")
                nc.scalar.copy(raw, ps)
                shf = rot_pool.tile([128, QC], f32, tag="shf")
                for dq, sq in ((0, 32), (32, 0), (64, 96), (96, 64)):
                    nc.sync.dma_start(out=shf[dq:dq + 32, :],
                                      in_=raw[sq:sq + 32, :])
                tr = rot_pool.tile([128, QC], f32, tag="tr")
                nc.vector.tensor_tensor(out=tr, in0=shf, in1=sin_sb[:, qs], op=mult)
                qk_slice = qk_sb[:, f, qs]
                nc.vector.tensor_tensor(out=qk_slice, in0=raw, in1=cos_sb[:, qs], op=mult)
                nc.vector.tensor_tensor(out=qk_slice, in0=qk_slice, in1=tr, op=add)
            # v for the s-tiles inside this chunk (seq-major, + ones col)
            for stl in range(QC // 128):
                st = sc * (QC // 128) + stl
                psv = ps_ctx.tile([128, HPC * D], f32, tag="ctx")
                for ht in range(HT):
                    nc.tensor.matmul(
                        psv, lhsT=mm(xt[:, ht, stl * 128:(stl + 1) * 128]),
                        rhs=mm(wv_sb[:, ht, :]),
                        start=(ht == 0), stop=(ht == HT - 1))
                nc.vector.tensor_copy(
                    v_sb[:, st, :, 0:D],
                    psv.rearrange("p (h d) -> p h d", h=HPC))
                nc.vector.memset(v_sb[:, st, :, D:D + 1], 1.0)
            if stop_after != "qkv":
                for kt in range(sc * (QC // 128), (sc + 1) * (QC // 128)):
                    attn_iteration(0, 0, kt, ctx00, drip=False)
        if stop_after != "qkv":
            normalize(0, 0, ctx00)

        if "dbg_qk" in t:
            nc.sync.dma_start(out=t["dbg_qk"], in_=qk_sb)
            nc.sync.dma_start(out=t["dbg_v"], in_=v_sb)
        if stop_after == "qkv":
            return

        # ---- phase 2+3: attention with out-projection interleaved ----
        # Out-proj work for q-chunk qc is drip-fed into the PE stream during
        # later attention (the PE has slack each kt iteration while ACT chews
        # on the exps), so ACT never starves at chunk boundaries.
        for qc in range(NQC):
            for p in range(2):
                if qc == 0 and p == 0:
                    continue  # interleaved into phase 1 above
                ctx_ps = [ps_ctx.tile([D + 1, QC], f32, tag="ctx",
                                      name=f"ctxps{p}_{qc}_{i}")
                          for i in range(2)]
                for kt in range(NT):
                    attn_iteration(p, qc, kt, ctx_ps)
                normalize(p, qc, ctx_ps)
            if stop_after == "attn":
                continue
            # queue this q-chunk's out-projection units (emitted during the
            # next chunk's attention; drained at the end)
            queue_outproj(qc)

        while pending_out:
            emit_outproj_unit()

        if "dbg_cT" in t:
            nc.sync.dma_start(out=t["dbg_cT"], in_=cT_sb)


def build(cfg: Cfg, dbg=False, stop_after=None):
    """Build + compile the per-core program. Returns (nc, input names)."""
    import concourse.tile as tile
    from concourse import bacc, mybir

    f32 = mybir.dt.float32
    dmm = {"bf16": mybir.dt.bfloat16, "fp32": f32, "fp32r": f32}[cfg.mm_dtype]

    nc = bacc.Bacc("TRN2", debug=False, enable_asserts=False,
                   target_bir_lowering=False)
    t = {}
    t["xT"] = nc.dram_tensor("xT", [cfg.H, cfg.S], dmm, kind="ExternalInput").ap()
    t["wqk"] = nc.dram_tensor("wqk", [cfg.H, 4 * 128], dmm, kind="ExternalInput").ap()
    t["wv"] = nc.dram_tensor("wv", [cfg.H, HPC * D], dmm, kind="ExternalInput").ap()
    t["wo"] = nc.dram_tensor("wo", [HPC * D, cfg.H], dmm, kind="ExternalInput").ap()
    t["cosT"] = nc.dram_tensor("cosT", [128, cfg.S], f32, kind="ExternalInput").ap()
    t["sinS"] = nc.dram_tensor("sinS", [128, cfg.S], f32, kind="ExternalInput").ap()
    t["mbias"] = nc.dram_tensor("mbias", [128, cfg.NT], f32, kind="ExternalInput").ap()
    t["out"] = nc.dram_tensor("out", [cfg.S, cfg.H], f32, kind="ExternalOutput").ap()
    if dbg:
        t["dbg_qk"] = nc.dram_tensor(
            "dbg_qk", [128, 4, cfg.S], dmm, kind="ExternalOutput").ap()
        t["dbg_v"] = nc.dram_tensor(
            "dbg_v", [128, cfg.NT, HPC, D + 1], dmm, kind="ExternalOutput").ap()
        t["dbg_cT"] = nc.dram_tensor(
            "dbg_cT", [128, 2, cfg.S], dmm, kind="ExternalOutput").ap()

    with tile.TileContext(nc) as tc:
        _emit(tc, t, cfg, stop_after=stop_after)
    nc.compile()
    return nc


# ----------------------------------------------------------------------------
# Host side: shard, run, gather
# ----------------------------------------------------------------------------

def rope_tables(S, dtype=np.float32):
    """cos/sin tables exactly as the reference builds them."""
    inv_freq = 1.0 / (10000.0 ** (np.arange(0, D, 2, dtype=np.float32) / D))
    tt = np.arange(S, dtype=np.float32)
    freqs = np.einsum("i,j->ij", tt, inv_freq)  # [S, D/2]
    emb = np.concatenate([freqs, freqs], axis=-1)  # [S, D]
    return np.cos(emb).astype(dtype), np.sin(emb).astype(dtype)


def device_rope_tables(S):
    """(cosT, sinS) in the [128, S] partition layout the kernel expects.

    cosT: cos table transposed, stacked twice (two heads per 128 partitions).
    sinS: sign-folded sin table (rotate_half sign folded in), transposed,
    stacked twice.
    """
    cos, sin = rope_tables(S)
    cosT = np.ascontiguousarray(np.tile(cos.T, (2, 1)))  # [128, S]
    sinT = sin.T.copy()  # [D, S]
    sinT[:D // 2, :] *= -1.0
    sinS = np.ascontiguousarray(np.tile(sinT, (2, 1)))  # [128, S], signed
    return cosT.astype(np.float32), sinS.astype(np.float32)


def make_in_maps(x, attention_mask, w_qkv, w_out, cfg: Cfg):
    """Build the 8 per-core input maps (numpy prep only)."""
    import ml_dtypes
    np_mm = {"bf16": np.dtype(ml_dtypes.bfloat16),
             "fp32": np.float32, "fp32r": np.float32}[cfg.mm_dtype]

    S, H = cfg.S, cfg.H
    cosT, sinS = device_rope_tables(S)

    in_maps = []
    for core in range(N_CORES):
        b, g = core // GROUPS, core % GROUPS
        heads = [g * HPC + j for j in range(HPC)]  # global head ids
        # Reference reshapes qkv to [B,S,16,192]: head h owns w_qkv columns
        # [h*3D, (h+1)*3D) as [q (D) | k (D) | v (D)].
        qcols = np.concatenate([np.arange(h * 3 * D, h * 3 * D + D) for h in heads])
        kcols = np.concatenate([np.arange(h * 3 * D + D, h * 3 * D + 2 * D) for h in heads])
        vcols = np.concatenate([np.arange(h * 3 * D + 2 * D, h * 3 * D + 3 * D) for h in heads])
        wqk = np.ascontiguousarray(
            np.concatenate([w_qkv[:, qcols], w_qkv[:, kcols]], axis=1)).astype(np_mm)
        wv = np.ascontiguousarray(w_qkv[:, vcols]).astype(np_mm)
        wo = np.ascontiguousarray(
            w_out[heads[0] * D:(heads[-1] + 1) * D, :]).astype(np_mm)
        xT = np.ascontiguousarray(np.asarray(x)[b].T).astype(np_mm)  # [H, S]
        mask = np.asarray(attention_mask)[b].astype(np.float32)  # [S]
        mbias = ((mask - 1.0) * 3e38).reshape(cfg.NT, 128).T.copy()  # [128, NT]
        in_maps.append({
            "xT": xT, "wqk": wqk, "wv": wv, "wo": wo,
            "cosT": cosT, "sinS": sinS, "mbias": mbias.astype(np.float32),
        })
    return in_maps


_PROG_CACHE = {}


def get_program(cfg: Cfg):
    if cfg not in _PROG_CACHE:
        _PROG_CACHE[cfg] = build(cfg)
    return _PROG_CACHE[cfg]


def run_sharded(x, attention_mask, w_qkv, w_out, trace=False, cfg: Cfg = None):
    """Run on the 8 NeuronCores; returns (full_output, BassKernelResults)."""
    from concourse import bass_utils

    cfg = cfg or Cfg()
    nc = get_program(cfg)
    in_maps = make_in_maps(x, attention_mask, w_qkv, w_out, cfg)
    res = bass_utils.run_bass_kernel_spmd(
        nc, in_maps, core_ids=list(range(N_CORES)), trace=trace)
    parts = [r["out"].astype(np.float64) for r in res.results]
    out = np.stack([
        sum(parts[b * GROUPS + g] for g in range(GROUPS))
        for b in range(B)
    ]).astype(np.float32)  # [B, S, H]
    return out, res


def kernel(x=None, attention_mask=None, w_qkv=None, w_out=None, **_ignored):
    out, _ = run_sharded(x, attention_mask, w_qkv, w_out, trace=False)
    return out
